# revision 18
# baseline (speedup 1.0000x reference)
"""Trainium2 Bass kernel for causal MHA with RoPE (nn_MHA_14164802142240).

Full-input contract: kernel(x, W_qkv, W_o) -> [B, S, E], distributed across
8 NeuronCores as (batch x head-group): core c handles batch c//4 and heads
(c%4)*4 .. (c%4)*4+3.  Each core computes its 4 heads' attention plus the
partial output projection over its W_o row block; the host sums the 4
head-group partials per batch (fp16 partials, fp32 sum).

v2 design (vs the 200us baseline):
- QKV phase is pipelined by seq-chunk: xT arrives via per-chunk 3D-pattern
  DMAs so matmuls start at ~2.5us and stream continuously (HAM stays warm);
  RoPE + per-head repack overlap the next chunk's matmuls.
- No duplicated Q/K rows: scores contract over 64 partitions (cycle count is
  free-dim-bound); repack volume halves.
- Causal-compacted scores: diagonal-straddling tiles write left-packed PSUM
  regions (no uninit strips, no memsets); PSUM is evacuated by vector/gpsimd
  copies into a per-(chunk,head) fp16 staging row, then ONE scalar exp
  instruction covers the whole row (16 exps total instead of 80) so the
  scalar engine stays under the PE roofline.
- Causal masking inside diagonal 128-blocks is a single constant [128,128]
  triangle multiply per block, after exp.
- Output written fp16 (halves tail DMA), proj interleaved into the head
  stream with 2-chunk lookahead so exp latency hides under scores matmuls.
"""

import numpy as np

B, S, E = 2, 2048, 1024
H, D = 16, 64
HG = 4          # heads per core
NCORES = 8
SC = 512        # q-chunk
NSC = S // SC   # 4
NST = S // 128  # 16 seq tiles
NE = E // 128   # 8 contraction chunks
VW = 66         # per-head V stationary width: 64 v cols + ones + pad
ESCALE = 0.125  # 1/sqrt(D)
EBIAS = -2.0    # exp(s*ESCALE + EBIAS); cancels in normalization

_COMPILED = None


def _build_bass():
    import concourse.bass as bass
    import concourse.mybir as mybir
    import concourse.tile as tile
    from concourse import bacc
    from contextlib import ExitStack

    f32 = mybir.dt.float32
    f16 = mybir.dt.float16
    Exp = mybir.ActivationFunctionType.Exp

    nc = bacc.Bacc("TRN2", target_bir_lowering=False, debug=False,
                   enable_asserts=False)

    # sc-major, (p,e)-row-ordered chunks: row sc*E + p*NE + e = x.T[e*128+p, sc*SC:]
    xT_d = nc.dram_tensor("xT", [NSC * E, SC], f16, kind="ExternalInput").ap()
    wqk_d = nc.dram_tensor("wqk", [E, 4 * 128], f16, kind="ExternalInput").ap()
    wv_d = nc.dram_tensor("wv", [E, HG * D], f16, kind="ExternalInput").ap()
    wo_d = nc.dram_tensor("wo", [HG * D, E], f16, kind="ExternalInput").ap()
    cs_d = nc.dram_tensor("cs", [128, 2 * S], f16, kind="ExternalInput").ap()
    tri_d = nc.dram_tensor("tri", [128, 256], f16, kind="ExternalInput").ap()
    out_d = nc.dram_tensor("out", [S, E], f16, kind="ExternalOutput").ap()

    with tile.TileContext(nc) as tc, ExitStack() as outer:
        pconst = outer.enter_context(tc.tile_pool(name="const", bufs=1))
        pv = outer.enter_context(tc.tile_pool(name="vbuf", bufs=1))
        pqk = outer.enter_context(tc.tile_pool(name="qkbuf", bufs=1))
        py = outer.enter_context(tc.tile_pool(name="ybuf", bufs=1))
        pp = outer.enter_context(tc.tile_pool(name="pbuf", bufs=4))
        psm = outer.enter_context(tc.tile_pool(name="small", bufs=3))
        ps_s = outer.enter_context(
            tc.tile_pool(name="ps_s", bufs=2, space="PSUM"))
        ps_y = outer.enter_context(
            tc.tile_pool(name="ps_y", bufs=1, space="PSUM"))

        tri_t = pconst.tile([128, 256], f16, tag="tri")
        ebias_t = pconst.tile([128, 1], f32, tag="ebias")
        nc.gpsimd.memset(ebias_t[:], EBIAS)
        wo_t = [pconst.tile([128, E], f16, name=f"wo{ft}", tag=f"wo{ft}")
                for ft in range(2)]

        vt = [pv.tile([128, HG * VW], f16, name=f"v{st}", tag=f"v{st}")
              for st in range(NST)]
        # per-head q/k, 64 rows (x0 pairs 0:32, x1 pairs 32:64)
        qd = [pqk.tile([64, S], f16, name=f"qd{i}", tag=f"qd{i}")
              for i in range(HG)]
        kd = [pqk.tile([64, S], f16, name=f"kd{i}", tag=f"kd{i}")
              for i in range(HG)]
        # unnormalized y^T (fp32) and normalized fp16 version for the proj
        yT = [py.tile([128, S], f32, name=f"y{i}", tag=f"y{i}")
              for i in range(2)]
        yT2 = [py.tile([128, S], f16, name=f"y2{i}", tag=f"y2{i}")
               for i in range(2)]

        LOOKP = 2

        def emit_slot(i, yeng):
            """One (chunk, head) attention slot: compacted scores pairs ->
            exp from PSUM -> merged triangle mask -> y matmuls (LOOKP
            behind) -> normalization.  yeng: engine for the v-psum-free
            copies (vector in ph2, scalar during ph1 interleave)."""
            c, h = i // HG, i % HG
            npair = 2 * c + 2
            nt = 4 * c + 4
            psy = ps_y.tile([128, SC], f32, tag="psy")
            pts = {}
            for pi in range(npair + LOOKP):
                if pi < npair:
                    blocks, dst = [], 0
                    for half in range(2):
                        t = 2 * pi + half
                        rg = max(0, 128 * (t - 4 * c))
                        blocks.append((dst, SC - rg, rg, t))
                        dst += SC - rg
                    wtot = dst
                    pss2 = ps_s.tile([128, 2 * SC], f32, tag="pss2")
                    pt = pp.tile([128, 2 * SC], f16, tag="pt")
                    for (dst, n, rg, t) in blocks:
                        nc.tensor.matmul(
                            pss2[:, dst:dst + n],
                            lhsT=kd[h][:, t * 128:(t + 1) * 128],
                            rhs=qd[h][:, c * SC + rg:(c + 1) * SC],
                            start=True, stop=True)
                    nc.scalar.activation(pt[:, 0:wtot], pss2[:, 0:wtot],
                                         Exp, bias=ebias_t[:], scale=ESCALE)
                    for (dst, n, rg, t) in blocks:
                        if t >= 4 * c:  # diagonal block: triangle mask
                            nc.vector.tensor_mul(pt[:, dst:dst + 128],
                                                 pt[:, dst:dst + 128],
                                                 tri_t[:, 0:128])
                    pts[pi] = (pt, blocks)
                pp_ = pi - LOOKP
                if 0 <= pp_ < npair:
                    pt, blocks = pts.pop(pp_)
                    for (dst, n, rg, t) in blocks:
                        nc.tensor.matmul(
                            psy[0:VW, rg:SC],
                            lhsT=vt[t][:, VW * h:VW * (h + 1)],
                            rhs=pt[:, dst:dst + n],
                            start=(t == 0), stop=(t == nt - 1))
            # normalization: denom -> recip -> broadcast -> scale
            # (partition-remapping copies: only DVE TensorCopy can do these)
            ro = 64 * (h % 2)
            nc.vector.tensor_copy(
                yT[h // 2][ro:ro + 64, c * SC:(c + 1) * SC], psy[0:D, :])
            lrow = psm.tile([1, SC], f32, tag="lrow")
            nc.vector.tensor_copy(lrow[:], psy[D:D + 1, :])
            rrow = psm.tile([1, SC], f32, tag="rrow")
            nc.vector.reciprocal_approx_fast(rrow[:], lrow[:])
            rbc = psm.tile([128, SC], f32, tag="rbc")
            nc.gpsimd.partition_broadcast(rbc[:], rrow[:])
            # norm-mul on vector: a gpsimd tensor op would force a pool
            # reconfig around every partition_broadcast (~7us each)
            nc.vector.tensor_mul(
                yT2[h // 2][ro:ro + 64, c * SC:(c + 1) * SC],
                yT[h // 2][ro:ro + 64, c * SC:(c + 1) * SC],
                rbc[ro:ro + 64, :])

        def yeng_copy(eng, dst, src):
            if eng is nc.scalar:
                nc.scalar.copy(dst, src)
            else:
                nc.vector.tensor_copy(dst, src)

        # ---------------- phase 1: QKV + RoPE + repack + c0/c1 attn ------
        with ExitStack() as ph1:
            px = ph1.enter_context(tc.tile_pool(name="xt", bufs=1))
            pw = ph1.enter_context(tc.tile_pool(name="w", bufs=1))
            pqkraw = ph1.enter_context(tc.tile_pool(name="qkraw", bufs=1))
            ptmp = ph1.enter_context(tc.tile_pool(name="ropetmp", bufs=2))
            ps1 = ph1.enter_context(
                tc.tile_pool(name="ps1", bufs=1, space="PSUM"))

            xt = px.tile([128, NE * S], f16, tag="xt")
            wqk_t = pw.tile([128, NE * 512], f16, tag="wqk")
            wv_t = pw.tile([128, NE * HG * D], f16, tag="wv")
            cs_t = pw.tile([128, 2 * S], f16, tag="cs")
            # x0 cols 0:S, x1 cols S:2S
            qkraw_q = pqkraw.tile([128, 2 * S], f16, tag="qkq")
            qkraw_k = pqkraw.tile([128, 2 * S], f16, tag="qkk")

            xt_v = xt.rearrange("p (e s) -> p e s", e=NE)
            wqk_v = wqk_t.rearrange("p (e c) -> p e c", e=NE)
            wqk_src = wqk_d.rearrange("(e p) c -> p e c", p=128)
            wv_src = wv_d.rearrange("(e p) c -> p e c", p=128)
            xT_src = xT_d.rearrange("(sc p e) s -> sc p e s", p=128, e=NE)

            # input DMA issue order: the first qk pass consumes wqk and
            # xt-sc0 e-block by e-block, so land them as interleaved 256KB
            # pieces round-robined over the three DMA queues
            dqs = [nc.sync, nc.gpsimd, nc.scalar]
            for eh in range(4):
                es = slice(2 * eh, 2 * eh + 2)
                dqs[(2 * eh) % 3].dma_start(wqk_v[:, es, :], wqk_src[:, es, :])
                dqs[(2 * eh + 1) % 3].dma_start(xt_v[:, es, 0:SC],
                                                xT_src[0, :, es, :])
            nc.scalar.dma_start(wv_t[:], wv_src)
            nc.scalar.dma_start(cs_t[:], cs_d)
            nc.gpsimd.dma_start(xt_v[:, :, SC:2 * SC], xT_src[1, :, :, :])
            nc.sync.dma_start(xt_v[:, :, 2 * SC:3 * SC], xT_src[2, :, :, :])
            nc.gpsimd.dma_start(xt_v[:, :, 3 * SC:4 * SC], xT_src[3, :, :, :])
            nc.sync.dma_start(tri_t[:], tri_d)
            for ft in range(2):
                nc.sync.dma_start(wo_t[ft][:], wo_d[ft * 128:(ft + 1) * 128, :])

            # ones/pad columns of v stationaries (during initial DMA wait)
            for st in range(NST):
                v_view = vt[st].rearrange("p (h w) -> p h w", h=HG)
                nc.gpsimd.memset(v_view[:, :, D:D + 1], 1.0)
                nc.gpsimd.memset(v_view[:, :, D + 1:VW], 0.0)

            cos = cs_t[:, 0:S]
            sin = cs_t[:, S:2 * S]

            def qk_pass(sc, jt, eng):
                """one jt block accumulated over e; evac into qkraw."""
                ps = ps1.tile([128, SC], f32, tag=f"p1{jt % 2}")
                for e in range(NE):
                    nc.tensor.matmul(
                        ps[:],
                        lhsT=wqk_t[:, e * 512 + jt * 128:
                                   e * 512 + (jt + 1) * 128],
                        rhs=xt[:, e * S + sc * SC:e * S + (sc + 1) * SC],
                        start=(e == 0), stop=(e == NE - 1))
                dstt = qkraw_q if jt < 2 else qkraw_k
                off = S * (jt % 2) + sc * SC
                yeng_copy(eng, dstt[:, off:off + SC], ps[:])

            def v_pass(sc, half, eng):
                """two st tiles, one accumulation group per PSUM tile
                (interleaved groups sharing a bank are broken in HW)."""
                for j2 in range(2):
                    st = 4 * sc + 2 * half + j2
                    ps = ps1.tile([128, HG * D], f32, tag=f"p1{j2}")
                    for e in range(NE):
                        nc.tensor.matmul(
                            ps[:],
                            lhsT=xt[:, e * S + st * 128:
                                    e * S + st * 128 + 128],
                            rhs=wv_t[:, e * 256:(e + 1) * 256],
                            start=(e == 0), stop=(e == NE - 1))
                    v_view = vt[st].rearrange("p (h w) -> p h w", h=HG)
                    yeng_copy(eng, v_view[:, :, 0:D],
                              ps.rearrange("p (h d) -> p h d", h=HG))

            def rope(qk, sc):
                t = qkraw_q if qk == 0 else qkraw_k
                x0 = t[:, sc * SC:(sc + 1) * SC]
                x1 = t[:, S + sc * SC:S + (sc + 1) * SC]
                c_ = cos[:, sc * SC:(sc + 1) * SC]
                s_ = sin[:, sc * SC:(sc + 1) * SC]
                tmp = ptmp.tile([128, SC], f16, tag="rt0")
                tmp2 = ptmp.tile([128, SC], f16, tag="rt1")
                nc.vector.tensor_mul(tmp[:], x0, s_)     # x0*sin
                nc.vector.tensor_mul(tmp2[:], x1, s_)    # x1*sin
                nc.vector.tensor_mul(x0, x0, c_)         # x0*cos
                nc.vector.tensor_mul(x1, x1, c_)         # x1*cos
                nc.vector.tensor_sub(x0, x0, tmp2[:])    # x0 c - x1 s
                nc.vector.tensor_add(x1, x1, tmp[:])     # x0 s + x1 c

            def repack(sc_hi):
                """repack the two chunks (sc_hi-1, sc_hi) into qd/kd."""
                pc = slice((sc_hi - 1) * SC, (sc_hi + 1) * SC)
                pcx1 = slice(S + (sc_hi - 1) * SC, S + (sc_hi + 1) * SC)
                qs = [nc.sync, nc.gpsimd, nc.sync, nc.gpsimd]
                for h in range(HG):
                    sl = slice(32 * h, 32 * h + 32)
                    qs[h % 4].dma_start(qd[h][0:32, pc], qkraw_q[sl, pc])
                    qs[(h + 1) % 4].dma_start(qd[h][32:64, pc],
                                              qkraw_q[sl, pcx1])
                    qs[(h + 2) % 4].dma_start(kd[h][0:32, pc],
                                              qkraw_k[sl, pc])
                    qs[(h + 3) % 4].dma_start(kd[h][32:64, pc],
                                              qkraw_k[sl, pcx1])

            V = nc.vector
            SCL = nc.scalar
            # sc0, sc1: pure QKV; repack pair0 at end of sc1
            for sc in (0, 1):
                qk_pass(sc, 0, V)
                qk_pass(sc, 1, V)
                rope(0, sc)
                qk_pass(sc, 2, V)
                qk_pass(sc, 3, V)
                rope(1, sc)
                v_pass(sc, 0, V)
                v_pass(sc, 1, V)
                if sc == 1:
                    repack(1)
            # sc2 with c0 attention slots woven in (v evacs on scalar to
            # keep vector within budget)
            qk_pass(2, 0, V)
            qk_pass(2, 1, V)
            rope(0, 2)
            emit_slot(0, SCL)
            qk_pass(2, 2, V)
            emit_slot(1, SCL)
            qk_pass(2, 3, V)
            rope(1, 2)
            emit_slot(2, SCL)
            v_pass(2, 0, SCL)
            emit_slot(3, SCL)
            v_pass(2, 1, SCL)
            # sc3 with c1 slots; repack pair1 as soon as RoPE(sc3) done
            qk_pass(3, 0, V)
            qk_pass(3, 1, V)
            rope(0, 3)
            qk_pass(3, 2, V)
            emit_slot(4, SCL)
            qk_pass(3, 3, V)
            rope(1, 3)
            repack(3)
            emit_slot(5, SCL)
            v_pass(3, 0, SCL)
            emit_slot(6, SCL)
            v_pass(3, 1, SCL)
            emit_slot(7, SCL)

        # ---------------- phase 2: c2/c3 attention + all projections -----
        with ExitStack() as ph2:
            pob = ph2.enter_context(tc.tile_pool(name="outbuf", bufs=4))
            ps_o = ph2.enter_context(
                tc.tile_pool(name="ps_o", bufs=2, space="PSUM"))

            ob_i = 0

            def emit_proj(c):
                nonlocal ob_i
                for st in range(4 * c, 4 * c + 4):
                    for ec in range(2):
                        pso = ps_o.tile([128, SC], f32, tag="pso")
                        for ft in range(2):
                            nc.tensor.matmul(
                                pso[:],
                                lhsT=yT2[ft][:, st * 128:(st + 1) * 128],
                                rhs=wo_t[ft][:, ec * SC:(ec + 1) * SC],
                                start=(ft == 0), stop=(ft == 1))
                        ob = pob.tile([128, SC], f16, tag="ob")
                        nc.scalar.copy(ob[:], pso[:])
                        (nc.sync if ob_i % 2 == 0 else nc.gpsimd).dma_start(
                            out_d[st * 128:(st + 1) * 128,
                                  ec * SC:(ec + 1) * SC],
                            ob[:])
                        ob_i += 1

            emit_slot(8, nc.vector)
            emit_proj(0)
            emit_slot(9, nc.vector)
            emit_slot(10, nc.vector)
            emit_proj(1)
            emit_slot(11, nc.vector)
            emit_slot(12, nc.vector)
            emit_proj(2)
            emit_slot(13, nc.vector)
            emit_slot(14, nc.vector)
            emit_slot(15, nc.vector)
            emit_proj(3)

    nc.compile()
    return nc


def _host_inputs(x, W_qkv, W_o):
    """Build the 8 per-core input maps (fp16 device-side compute dtypes)."""
    thetas = 10000.0 ** (-2.0 * (np.arange(D // 2, dtype=np.float32) / D))
    freqs = np.arange(S, dtype=np.float32)[:, None] * thetas[None, :]  # [S, 32]
    cosT = np.cos(freqs).astype(np.float32).T  # [32, S]
    sinT = np.sin(freqs).astype(np.float32).T
    cs = np.ascontiguousarray(np.concatenate(
        [np.tile(cosT, (4, 1)), np.tile(sinT, (4, 1))], axis=1)
        .astype(np.float16))  # [128, 2S]

    jj = np.arange(128)[:, None]
    tri1 = (jj <= np.arange(128)[None, :]).astype(np.float16)  # [128, 128]
    tri = np.ascontiguousarray(np.concatenate([tri1, tri1], axis=1))

    # [4*E, SC]: chunk sc contiguous, rows (p, e)-ordered to match the
    # SBUF xt view [128 p, 8 e, 512]
    xTs = [np.ascontiguousarray(
        x[b].T.astype(np.float16).reshape(NE, 128, NSC, SC)
        .transpose(2, 1, 0, 3).reshape(NSC * E, SC)) for b in range(B)]

    in_maps = []
    for core in range(NCORES):
        b, hg = core // 4, core % 4
        heads = range(hg * HG, (hg + 1) * HG)
        qx0 = [h * D + 2 * m for h in heads for m in range(D // 2)]
        qx1 = [h * D + 2 * m + 1 for h in heads for m in range(D // 2)]
        rows = (qx0 + qx1 + [E + i for i in qx0] + [E + i for i in qx1])
        wqk = np.ascontiguousarray(W_qkv[rows].T.astype(np.float16))  # [E, 512]
        vrows = [2 * E + h * D + d for h in heads for d in range(D)]
        wv = np.ascontiguousarray(W_qkv[vrows].T.astype(np.float16))  # [E, 256]
        wo = np.ascontiguousarray(
            W_o[:, hg * HG * D:(hg + 1) * HG * D].T.astype(np.float16))
        in_maps.append({
            "xT": xTs[b], "wqk": wqk, "wv": wv, "wo": wo,
            "cs": cs, "tri": tri,
        })
    return in_maps


def kernel(x, W_qkv, W_o):
    global _COMPILED
    x = np.ascontiguousarray(np.asarray(x, dtype=np.float32))
    W_qkv = np.ascontiguousarray(np.asarray(W_qkv, dtype=np.float32))
    W_o = np.ascontiguousarray(np.asarray(W_o, dtype=np.float32))

    if _COMPILED is None:
        _COMPILED = _build_bass()
    nc = _COMPILED

    from concourse.bass_utils import run_bass_kernel_spmd
    in_maps = _host_inputs(x, W_qkv, W_o)
    res = run_bass_kernel_spmd(nc, in_maps, core_ids=list(range(NCORES)))
    out = np.zeros((B, S, E), dtype=np.float32)
    for core in range(NCORES):
        out[core // 4] += res.results[core]["out"].astype(np.float32)
    return out
